# revision 17
# baseline (speedup 1.0000x reference)
"""BitLinear (per-token int8 activation quant + ternary weight quant + matmul)
as a Bass/Tile kernel on 8 Trainium2 NeuronCores.

Strategy (data-parallel tokens + tensor-parallel weight-mean + slab rotation):
  - x [4,2048,4096] -> [8192,4096]; each core quantizes and matmuls its own
    1024-token slab against the FULL weight; outputs concatenate on tokens.
  - Each core receives W pre-rolled by 512*i rows (host-side np.roll, pure
    layout). Rows [0:512) of its (rolled) W are both its 1/8 mean-shard AND
    the weights of its first two output slabs. A 512B AllReduce combines the
    per-core |W| partial sums. The host un-rolls the output columns after.
  - q = rint(x*s) and tw in {-1,0,1} are exact in bf16 => the bf16 matmul
    with fp32 PSUM accumulation is EXACT integer arithmetic; per-token
    dequant scales applied on the PSUM->SBUF copy.
  - Engine split: gpsimd does the whole mean/AllReduce chain + half of the
    elementwise quant/ternarize work + qT transposes + output stores; vector
    does the other half; scalar ring only streams W; sync ring streams x and
    the twT transposes. W is processed in [128,2048] half-blocks for finer
    pipelining; ternarize is 3 pure-vector ops (scale+magic-round fold).
  - OF_CHUNK=256 (16 slabs), ternarize runs two slabs ahead of the matmul.
"""
import numpy as np
from ml_dtypes import bfloat16
from contextlib import ExitStack

N_CORES = 8
B, S, D_IN, D_OUT = 4, 2048, 4096, 4096
TOK = B * S                  # 8192
TOK_PC = TOK // N_CORES      # 1024 tokens per core
N_TOK_TILES = TOK_PC // 128  # 8
N_K = D_IN // 128            # 32 contraction tiles
OF_CHUNK = 256
N_SLAB = D_OUT // OF_CHUNK   # 16
SHARD_ROWS = D_OUT // N_CORES  # 512 weight rows per core for the mean
HD = D_IN // 2               # 2048 half-row
EPS = 1e-5
MAGIC = float(np.float32(1.5 * 2 ** 23))   # fp32 round-to-nearest-even trick
MEAN_SCALE = float(np.float32(1.0 / (D_IN * D_OUT)))  # 2^-24, exact
INV127 = float(np.float32(1.0 / 127.0))

_CACHE = {}


def _build_module():
    import concourse.bacc as bacc
    import concourse.tile as tile
    import concourse.mybir as mybir
    import concourse.bass_isa as bass_isa

    dt = mybir.dt
    AF = mybir.ActivationFunctionType
    AL = mybir.AluOpType
    AX = mybir.AxisListType

    nc = bacc.Bacc(
        "TRN2", target_bir_lowering=False, debug=False, num_devices=N_CORES
    )
    xs = nc.dram_tensor("xs", [TOK_PC, D_IN], dt.bfloat16, kind="ExternalInput").ap()
    wf = nc.dram_tensor("wf", [D_OUT, D_IN], dt.float32, kind="ExternalInput").ap()
    out = nc.dram_tensor("out", [TOK_PC, D_OUT], dt.bfloat16, kind="ExternalOutput").ap()

    with tile.TileContext(nc) as tc, ExitStack() as ctx:
        stats = ctx.enter_context(tc.tile_pool(name="stats", bufs=1))
        qT_pool = ctx.enter_context(tc.tile_pool(name="qT", bufs=N_TOK_TILES))
        big = ctx.enter_context(tc.tile_pool(name="big", bufs=6))
        qb_pool = ctx.enter_context(tc.tile_pool(name="qbp", bufs=4))
        twc_pool = ctx.enter_context(tc.tile_pool(name="twcp", bufs=3))
        twTp = ctx.enter_context(tc.tile_pool(name="twT", bufs=2))
        op = ctx.enter_context(tc.tile_pool(name="op", bufs=6))
        pp = ctx.enter_context(tc.tile_pool(name="pp", bufs=7, space="PSUM"))
        dram = ctx.enter_context(tc.tile_pool(name="dram", bufs=2, space="DRAM"))

        amc = stats.tile([128, N_TOK_TILES], dt.float32, tag="amc")
        am2 = stats.tile([128, N_TOK_TILES], dt.float32, tag="am2")
        sca = stats.tile([128, N_TOK_TILES], dt.float32, tag="sca")
        dq = stats.tile([128, N_TOK_TILES], dt.float32, tag="dq")
        wme = stats.tile([128, 1], dt.float32, tag="wme")
        swt = stats.tile([128, 1], dt.float32, tag="swt")
        wp = stats.tile([128, 8], dt.float32, tag="wp")
        w16 = stats.tile([128, 16], dt.float32, tag="w16")
        zr = stats.tile([128, 1], dt.float32, tag="zr")
        gtot = stats.tile([128, 1], dt.float32, tag="gtot")
        gl = stats.tile([128, 1], dt.float32, tag="gl")
        xsc = stats.tile([128, HD], dt.float32, tag="xsc")
        xsc2 = stats.tile([128, HD], dt.float32, tag="xsc2")

        arin = dram.tile([128, 1], dt.float32, tag="arin")
        arout = dram.tile([128, 1], dt.float32, tag="arout")

        # ---- |W| mean shard = rows [0:512) of the rolled W (gpsimd) ----
        # blocks 0-2 stay resident as 6 half-tiles (they are also slab-0/1
        # weights); block 3 is reduced from throwaway halves and re-read.
        shard_halves = []
        with nc.named_scope("wmean"), tc.tile_pool(name="xq0", bufs=2) as xq0:
            for hh in range(8):
                blk, h = hh // 2, hh % 2
                if blk < 3:
                    wt = big.tile([128, HD], dt.float32, tag="big", name=f"sh{hh}")
                    shard_halves.append(wt)
                else:
                    wt = xq0.tile([128, HD], dt.float32, tag="xq", name=f"s3_{h}")
                nc.scalar.dma_start(
                    wt[:], wf[blk * 128:(blk + 1) * 128, h * HD:(h + 1) * HD]
                )
                nc.vector.tensor_reduce(
                    w16[:],
                    wt[:].rearrange("p (a b) -> p a b", b=128),
                    axis=AX.X, op=AL.add, apply_absolute_value=True,
                )
                nc.vector.tensor_reduce(
                    wp[:, hh:hh + 1], w16[:], axis=AX.X, op=AL.add
                )
            nc.vector.tensor_reduce(zr[:], wp[:], axis=AX.X, op=AL.add)
            nc.gpsimd.partition_all_reduce(
                gtot[:], zr[:], channels=128, reduce_op=bass_isa.ReduceOp.add
            )
            nc.gpsimd.dma_start(arin[:], gtot[:])
            nc.gpsimd.collective_compute(
                "AllReduce",
                mybir.AluOpType.add,
                replica_groups=[list(range(N_CORES))],
                ins=[arin.opt()],
                outs=[arout.opt()],
            )

        # ---- x-quant: own tokens -> resident qT tiles (AR-independent) ----
        # vector handles half 0, gpsimd half 1; qT transposes on gpsimd ring.
        qT_tiles = []
        with nc.named_scope("xquant"), tc.tile_pool(name="xq", bufs=3) as xq:
            for t in range(N_TOK_TILES):
                qT_t = qT_pool.tile(
                    [128, N_K, 128], dt.bfloat16, tag="qT", name=f"qT{t}"
                )
                xh = []
                for h in range(2):
                    xth = xq.tile([128, HD], dt.bfloat16, tag="xq", name=f"xt{t}_{h}")
                    nc.sync.dma_start(
                        xth[:], xs[t * 128:(t + 1) * 128, h * HD:(h + 1) * HD]
                    )
                    nc.vector.tensor_reduce(
                        (amc if h == 0 else am2)[:, t:t + 1],
                        xth[:], axis=AX.X, op=AL.max, apply_absolute_value=True,
                    )
                    xh.append(xth)
                # amax = max(half0, half1, EPS); s = 127/amax
                nc.vector.tensor_tensor(
                    amc[:, t:t + 1], amc[:, t:t + 1], am2[:, t:t + 1], op=AL.max
                )
                nc.vector.tensor_scalar(
                    amc[:, t:t + 1], amc[:, t:t + 1], EPS, None, op0=AL.max
                )
                nc.vector.reciprocal(sca[:, t:t + 1], amc[:, t:t + 1])
                nc.vector.tensor_scalar(
                    sca[:, t:t + 1], sca[:, t:t + 1], 127.0, None, op0=AL.mult
                )
                for h in range(2):
                    scr = xsc if h == 0 else xsc2
                    qbh = qb_pool.tile(
                        [128, HD], dt.bfloat16, tag="qb", name=f"qb{t}_{h}"
                    )
                    # q = rint(x*s): fp32 magic-number round
                    # scalar engine: scr = x*s + MAGIC; vector: qb = scr - MAGIC
                    nc.scalar.activation(
                        scr[:], xh[h][:], AF.Copy,
                        bias=MAGIC, scale=sca[:, t:t + 1],
                    )
                    nc.vector.tensor_scalar(
                        qbh[:], scr[:], MAGIC, None, op0=AL.subtract,
                    )
                    nc.scalar.dma_start(
                        qT_t[:, h * (N_K // 2):(h + 1) * (N_K // 2), :],
                        qbh[:], transpose=True,
                    )
                qT_tiles.append(qT_t)

        # ---- AR-dependent epilogue of the mean (vector) ----
        with nc.named_scope("wmean"):
            nc.gpsimd.dma_start(gl[:], arout[:])
            nc.vector.tensor_scalar(
                wme[:], gl[:], MEAN_SCALE, EPS, op0=AL.mult, op1=AL.max
            )
            nc.vector.reciprocal(swt[:], wme[:])
            # per-token dequant scale: amax * mean|W| / 127
            nc.vector.tensor_scalar(
                dq[:], amc[:], wme[:, 0:1], INV127, op0=AL.mult, op1=AL.mult
            )

        # ---- per-slab: ternarize ~two slabs ahead, then matmul ----
        def stage_tern(c):
            twT_c = twTp.tile(
                [128, N_K, OF_CHUNK], dt.bfloat16, tag="twT", name=f"twT{c}"
            )
            for j in range(2):
                blk = 2 * c + j
                for h in range(2):
                    if blk < 3:
                        wh = shard_halves[blk * 2 + h]
                    else:
                        wh = big.tile(
                            [128, HD], dt.float32, tag="big", name=f"wt{blk}_{h}"
                        )
                        nc.scalar.dma_start(
                            wh[:],
                            wf[blk * 128:(blk + 1) * 128, h * HD:(h + 1) * HD],
                        )
                    # v = W*swt + MAGIC  (exact rint encoding in the mantissa)
                    nc.scalar.activation(
                        wh[:], wh[:], AF.Copy, bias=MAGIC, scale=swt[:, 0:1]
                    )
                    tch = twc_pool.tile(
                        [128, HD], dt.bfloat16, tag="twc", name=f"twc{blk}_{h}"
                    )
                    # rint(W*swt) to bf16 (exact, values are small ints)...
                    nc.vector.tensor_scalar(
                        tch[:], wh[:], MAGIC, None, op0=AL.subtract
                    )
                    # ...then ternary clip on bf16 (2x DVE rate)
                    nc.vector.tensor_scalar(
                        tch[:], tch[:], 1.0, -1.0, op0=AL.min, op1=AL.max
                    )
                    nc.sync.dma_start(
                        twT_c[:, h * (N_K // 2):(h + 1) * (N_K // 2),
                              j * 128:(j + 1) * 128],
                        tch[:], transpose=True,
                    )
            return twT_c

        def stage_mm(c, twT_c):
            for t in range(N_TOK_TILES):
                ps = pp.tile([128, OF_CHUNK], dt.float32, tag="ps", name=f"ps{c}_{t}")
                for k in range(N_K):
                    nc.tensor.matmul(
                        ps[:], qT_tiles[t][:, k, :], twT_c[:, k, :],
                        start=(k == 0), stop=(k == N_K - 1),
                    )
                ot = op.tile([128, OF_CHUNK], dt.bfloat16, tag="ot", name=f"ot{c}_{t}")
                nc.vector.tensor_scalar(
                    ot[:], ps[:], dq[:, t:t + 1], None, op0=AL.mult
                )
                nc.gpsimd.dma_start(
                    out[t * 128:(t + 1) * 128, c * OF_CHUNK:(c + 1) * OF_CHUNK],
                    ot[:],
                )

        with nc.named_scope("mm"):
            pending = [stage_tern(0), stage_tern(1)]
            for c in range(N_SLAB):
                stage_mm(c, pending.pop(0))
                if c + 2 < N_SLAB:
                    pending.append(stage_tern(c + 2))

    nc.compile()
    return nc


def _get_module():
    if "nc" not in _CACHE:
        _CACHE["nc"] = _build_module()
    return _CACHE["nc"]


def _make_in_maps(x2, w2):
    # core i gets W rolled so its mean-shard == its first two slabs' rows
    return [
        {
            "xs": x2[i * TOK_PC:(i + 1) * TOK_PC].astype(bfloat16),
            "wf": np.ascontiguousarray(
                np.roll(w2, -SHARD_ROWS * i, axis=0)
            ) if i else w2,
        }
        for i in range(N_CORES)
    ]


def kernel(x: np.ndarray, weight: np.ndarray) -> np.ndarray:
    from concourse.bass_utils import run_bass_kernel_spmd

    x = np.asarray(x, dtype=np.float32)
    weight = np.asarray(weight, dtype=np.float32)
    x2 = np.ascontiguousarray(x.reshape(TOK, D_IN))
    w2 = np.ascontiguousarray(weight)

    in_maps = _make_in_maps(x2, w2)
    nc = _get_module()
    res = run_bass_kernel_spmd(nc, in_maps, list(range(N_CORES)))
    # core i's output columns are rolled by -512*i (it computed the rolled
    # weight rows in order); roll them back before concatenating tokens
    parts = [
        np.roll(np.asarray(res.results[i]["out"], dtype=np.float32),
                SHARD_ROWS * i, axis=1) if i
        else np.asarray(res.results[i]["out"], dtype=np.float32)
        for i in range(N_CORES)
    ]
    out = np.concatenate(parts, axis=0)
    return out.reshape(B, S, D_OUT)


# revision 19
# speedup vs baseline: 1.0447x; 1.0447x over previous
"""BitLinear (per-token int8 activation quant + ternary weight quant + matmul)
as a Bass/Tile kernel on 8 Trainium2 NeuronCores.

Strategy (data-parallel tokens + tensor-parallel weight-mean + slab rotation):
  - x [4,2048,4096] -> [8192,4096]; each core quantizes and matmuls its own
    1024-token slab against the FULL weight; outputs concatenate on tokens.
  - Each core receives W pre-rolled by 512*i rows (host-side np.roll, pure
    layout). Rows [0:512) of its (rolled) W are both its 1/8 mean-shard AND
    the weights of its first two output slabs. A 512B AllReduce combines the
    per-core |W| partial sums. The host un-rolls the output columns after.
  - The host ALSO ships x transposed (xt [4096, 1024] bf16, pure layout):
    q is generated directly in the transposed layout the PE needs, so the
    kernel needs NO DMA transposes before the AllReduce completes (in-flight
    collectives serialize all xbar transposes -- measured on hw).  Per-token
    scales reach the transposed layout via a small DRAM bounce-gather plus a
    gpsimd partition_broadcast.
  - q = rint(x*s) and tw in {-1,0,1} are exact in bf16 => the bf16 matmul
    with fp32 PSUM accumulation is EXACT integer arithmetic; per-token
    dequant scales applied on the PSUM->SBUF copy.
  - Engine split: vector does amax/q-gen/ternarize-finish/dequant; scalar
    does the W*swt+MAGIC activation and streams W; sync streams x/xt and the
    (post-AR) twT transposes; gpsimd does the AllReduce chain, broadcasts
    and output stores.  W is ternarized in [128,2048] half-blocks.
  - OF_CHUNK=256 (16 slabs), ternarize runs two slabs ahead of the matmul.
"""
import numpy as np
from ml_dtypes import bfloat16
from contextlib import ExitStack

N_CORES = 8
B, S, D_IN, D_OUT = 4, 2048, 4096, 4096
TOK = B * S                  # 8192
TOK_PC = TOK // N_CORES      # 1024 tokens per core
N_TOK_TILES = TOK_PC // 128  # 8
N_K = D_IN // 128            # 32 contraction tiles
OF_CHUNK = 256
N_SLAB = D_OUT // OF_CHUNK   # 16
SHARD_ROWS = D_OUT // N_CORES  # 512 weight rows per core for the mean
HD = D_IN // 2               # 2048 half-row
HT = TOK_PC // 2             # 512 half of the tokens
EPS = 1e-5
MAGIC = float(np.float32(1.5 * 2 ** 23))   # fp32 round-to-nearest-even trick
MEAN_SCALE = float(np.float32(1.0 / (D_IN * D_OUT)))  # 2^-24, exact
INV127 = float(np.float32(1.0 / 127.0))

_CACHE = {}


def _build_module():
    import concourse.bacc as bacc
    import concourse.tile as tile
    import concourse.mybir as mybir
    import concourse.bass_isa as bass_isa

    dt = mybir.dt
    AF = mybir.ActivationFunctionType
    AL = mybir.AluOpType
    AX = mybir.AxisListType

    nc = bacc.Bacc(
        "TRN2", target_bir_lowering=False, debug=False, num_devices=N_CORES
    )
    xs = nc.dram_tensor("xs", [TOK_PC, D_IN], dt.bfloat16, kind="ExternalInput").ap()
    xt = nc.dram_tensor("xt", [D_IN, TOK_PC], dt.bfloat16, kind="ExternalInput").ap()
    wf = nc.dram_tensor("wf", [D_OUT, D_IN], dt.float32, kind="ExternalInput").ap()
    out = nc.dram_tensor("out", [TOK_PC, D_OUT], dt.bfloat16, kind="ExternalOutput").ap()

    with tile.TileContext(nc) as tc, ExitStack() as ctx:
        stats = ctx.enter_context(tc.tile_pool(name="stats", bufs=1))
        qT_pool = ctx.enter_context(tc.tile_pool(name="qT", bufs=N_K))
        big = ctx.enter_context(tc.tile_pool(name="big", bufs=6))
        xtp = ctx.enter_context(tc.tile_pool(name="xtp", bufs=6))
        twc_pool = ctx.enter_context(tc.tile_pool(name="twcp", bufs=3))
        twTp = ctx.enter_context(tc.tile_pool(name="twT", bufs=3))
        op = ctx.enter_context(tc.tile_pool(name="op", bufs=6))
        pp = ctx.enter_context(tc.tile_pool(name="pp", bufs=7, space="PSUM"))
        dram = ctx.enter_context(tc.tile_pool(name="dram", bufs=2, space="DRAM"))

        amc = stats.tile([128, N_TOK_TILES], dt.float32, tag="amc")
        am2 = stats.tile([128, N_TOK_TILES], dt.float32, tag="am2")
        sca = stats.tile([128, N_TOK_TILES], dt.float32, tag="sca")
        dq = stats.tile([128, N_TOK_TILES], dt.float32, tag="dq")
        wme = stats.tile([128, 1], dt.float32, tag="wme")
        swt = stats.tile([128, 1], dt.float32, tag="swt")
        wp = stats.tile([128, 8], dt.float32, tag="wp")
        w16 = stats.tile([128, 16], dt.float32, tag="w16")
        zr = stats.tile([128, 1], dt.float32, tag="zr")
        gtot = stats.tile([128, 1], dt.float32, tag="gtot")
        gl = stats.tile([128, 1], dt.float32, tag="gl")
        scaT = stats.tile([1, TOK_PC], dt.float32, tag="scaT")
        scaB = stats.tile([128, TOK_PC], dt.float32, tag="scaB")
        vscr = stats.tile([128, HT], dt.float32, tag="vscr")

        arin = dram.tile([128, 1], dt.float32, tag="arin")
        arout = dram.tile([128, 1], dt.float32, tag="arout")
        scad = [
            dram.tile([128, 4], dt.float32, tag="scad", name=f"scad{h}")
            for h in range(2)
        ]

        # ---- |W| mean shard = rows [0:512) of the rolled W ----
        # blocks 0-2 stay resident as 6 half-tiles (they are also slab-0/1
        # weights); block 3 is reduced from throwaway halves and re-read.
        shard_halves = []
        with nc.named_scope("wmean"), tc.tile_pool(name="xq0", bufs=2) as xq0:
            for hh in range(8):
                blk, h = hh // 2, hh % 2
                if blk < 3:
                    wt = big.tile([128, HD], dt.float32, tag="big", name=f"sh{hh}")
                    shard_halves.append(wt)
                else:
                    wt = xq0.tile([128, HD], dt.float32, tag="xq", name=f"s3_{h}")
                nc.scalar.dma_start(
                    wt[:], wf[blk * 128:(blk + 1) * 128, h * HD:(h + 1) * HD]
                )
                nc.vector.tensor_reduce(
                    w16[:],
                    wt[:].rearrange("p (a b) -> p a b", b=128),
                    axis=AX.X, op=AL.add, apply_absolute_value=True,
                )
                nc.vector.tensor_reduce(
                    wp[:, hh:hh + 1], w16[:], axis=AX.X, op=AL.add
                )
            nc.vector.tensor_reduce(zr[:], wp[:], axis=AX.X, op=AL.add)
            nc.gpsimd.partition_all_reduce(
                gtot[:], zr[:], channels=128, reduce_op=bass_isa.ReduceOp.add
            )
            nc.gpsimd.dma_start(arin[:], gtot[:])
            nc.gpsimd.collective_compute(
                "AllReduce",
                mybir.AluOpType.add,
                replica_groups=[list(range(N_CORES))],
                ins=[arin.opt()],
                outs=[arout.opt()],
            )

        # ---- x amax (row-major x) -> per-token scales (AR-independent) ----
        with nc.named_scope("xquant"), tc.tile_pool(name="xq", bufs=3) as xq:
            for t in range(N_TOK_TILES):
                for h in range(2):
                    xth = xq.tile([128, HD], dt.bfloat16, tag="xq", name=f"xt{t}_{h}")
                    nc.sync.dma_start(
                        xth[:], xs[t * 128:(t + 1) * 128, h * HD:(h + 1) * HD]
                    )
                    nc.vector.tensor_reduce(
                        (amc if h == 0 else am2)[:, t:t + 1],
                        xth[:], axis=AX.X, op=AL.max, apply_absolute_value=True,
                    )
                # amax = max(half0, half1, EPS); s = 127/amax
                nc.vector.tensor_tensor(
                    amc[:, t:t + 1], amc[:, t:t + 1], am2[:, t:t + 1], op=AL.max
                )
                nc.vector.tensor_scalar(
                    amc[:, t:t + 1], amc[:, t:t + 1], EPS, None, op0=AL.max
                )
                nc.vector.reciprocal(sca[:, t:t + 1], amc[:, t:t + 1])
                nc.vector.tensor_scalar(
                    sca[:, t:t + 1], sca[:, t:t + 1], 127.0, None, op0=AL.mult
                )
                # after each half of the token tiles: bounce the scales
                # through DRAM into token-major [1, 512] and broadcast to all
                # partitions (plain DMAs only -- no xbar transpose).
                if t == 3 or t == 7:
                    hb = t // 4
                    nc.scalar.dma_start(
                        scad[hb][:], sca[:, hb * 4:(hb + 1) * 4]
                    )
                    nc.scalar.dma_start(
                        scaT[0:1, hb * HT:(hb + 1) * HT],
                        scad[hb][:].rearrange("p c -> c p"),
                    )
                    nc.gpsimd.partition_broadcast(
                        scaB[:, hb * HT:(hb + 1) * HT],
                        scaT[0:1, hb * HT:(hb + 1) * HT],
                    )

            # ---- q-gen directly in transposed layout ----
            qT_tiles = [
                qT_pool.tile([128, TOK_PC], dt.bfloat16, tag="qT", name=f"qT{k}")
                for k in range(N_K)
            ]
            for hb in range(2):
                cs = slice(hb * HT, (hb + 1) * HT)
                for k in range(N_K):
                    xtk = xtp.tile([128, HT], dt.bfloat16, tag="xt", name=f"x{k}_{hb}")
                    nc.sync.dma_start(
                        xtk[:], xt[k * 128:(k + 1) * 128, cs]
                    )
                    nc.vector.tensor_tensor(
                        vscr[:], xtk[:], scaB[:, cs], op=AL.mult
                    )
                    nc.vector.tensor_scalar(
                        qT_tiles[k][:, cs], vscr[:], MAGIC, MAGIC,
                        op0=AL.add, op1=AL.subtract,
                    )

        # ---- AR-dependent epilogue of the mean (vector) ----
        with nc.named_scope("wmean"):
            nc.gpsimd.dma_start(gl[:], arout[:])
            nc.vector.tensor_scalar(
                wme[:], gl[:], MEAN_SCALE, EPS, op0=AL.mult, op1=AL.max
            )
            nc.vector.reciprocal(swt[:], wme[:])
            # per-token dequant scale: amax * mean|W| / 127
            nc.vector.tensor_scalar(
                dq[:], amc[:], wme[:, 0:1], INV127, op0=AL.mult, op1=AL.mult
            )

        # ---- per-slab: ternarize ~two slabs ahead, then matmul ----
        def stage_tern(c):
            twT_c = twTp.tile(
                [128, N_K, OF_CHUNK], dt.bfloat16, tag="twT", name=f"twT{c}"
            )
            for j in range(2):
                blk = 2 * c + j
                for h in range(2):
                    if blk < 3:
                        wh = shard_halves[blk * 2 + h]
                    else:
                        wh = big.tile(
                            [128, HD], dt.float32, tag="big", name=f"wt{blk}_{h}"
                        )
                        nc.scalar.dma_start(
                            wh[:],
                            wf[blk * 128:(blk + 1) * 128, h * HD:(h + 1) * HD],
                        )
                    # v = W*swt + MAGIC  (exact rint encoding in the mantissa)
                    nc.scalar.activation(
                        wh[:], wh[:], AF.Copy, bias=MAGIC, scale=swt[:, 0:1]
                    )
                    tch = twc_pool.tile(
                        [128, HD], dt.bfloat16, tag="twc", name=f"twc{blk}_{h}"
                    )
                    # rint(W*swt) to bf16 (exact, values are small ints)...
                    nc.vector.tensor_scalar(
                        tch[:], wh[:], MAGIC, None, op0=AL.subtract
                    )
                    # ...then ternary clip on bf16 (2x DVE rate)
                    nc.vector.tensor_scalar(
                        tch[:], tch[:], 1.0, -1.0, op0=AL.min, op1=AL.max
                    )
                    nc.sync.dma_start(
                        twT_c[:, h * (N_K // 2):(h + 1) * (N_K // 2),
                              j * 128:(j + 1) * 128],
                        tch[:], transpose=True,
                    )
            return twT_c

        def stage_mm(c, twT_c):
            for t in range(N_TOK_TILES):
                ps = pp.tile([128, OF_CHUNK], dt.float32, tag="ps", name=f"ps{c}_{t}")
                for k in range(N_K):
                    nc.tensor.matmul(
                        ps[:], qT_tiles[k][:, t * 128:(t + 1) * 128],
                        twT_c[:, k, :],
                        start=(k == 0), stop=(k == N_K - 1),
                    )
                ot = op.tile([128, OF_CHUNK], dt.bfloat16, tag="ot", name=f"ot{c}_{t}")
                nc.vector.tensor_scalar(
                    ot[:], ps[:], dq[:, t:t + 1], None, op0=AL.mult
                )
                nc.gpsimd.dma_start(
                    out[t * 128:(t + 1) * 128, c * OF_CHUNK:(c + 1) * OF_CHUNK],
                    ot[:],
                )

        with nc.named_scope("mm"):
            pending = [stage_tern(0), stage_tern(1)]
            for c in range(N_SLAB):
                stage_mm(c, pending.pop(0))
                if c + 2 < N_SLAB:
                    pending.append(stage_tern(c + 2))

    nc.compile()
    return nc


def _get_module():
    if "nc" not in _CACHE:
        _CACHE["nc"] = _build_module()
    return _CACHE["nc"]


def _make_in_maps(x2, w2):
    # core i gets W rolled so its mean-shard == its first two slabs' rows;
    # x is shipped both row-major (amax) and transposed (q-gen), bf16.
    maps = []
    for i in range(N_CORES):
        xsl = x2[i * TOK_PC:(i + 1) * TOK_PC].astype(bfloat16)
        maps.append({
            "xs": xsl,
            "xt": np.ascontiguousarray(xsl.T),
            "wf": np.ascontiguousarray(
                np.roll(w2, -SHARD_ROWS * i, axis=0)
            ) if i else w2,
        })
    return maps


def kernel(x: np.ndarray, weight: np.ndarray) -> np.ndarray:
    from concourse.bass_utils import run_bass_kernel_spmd

    x = np.asarray(x, dtype=np.float32)
    weight = np.asarray(weight, dtype=np.float32)
    x2 = np.ascontiguousarray(x.reshape(TOK, D_IN))
    w2 = np.ascontiguousarray(weight)

    in_maps = _make_in_maps(x2, w2)
    nc = _get_module()
    res = run_bass_kernel_spmd(nc, in_maps, list(range(N_CORES)))
    # core i's output columns are rolled by -512*i (it computed the rolled
    # weight rows in order); roll them back before concatenating tokens
    parts = [
        np.roll(np.asarray(res.results[i]["out"], dtype=np.float32),
                SHARD_ROWS * i, axis=1) if i
        else np.asarray(res.results[i]["out"], dtype=np.float32)
        for i in range(N_CORES)
    ]
    out = np.concatenate(parts, axis=0)
    return out.reshape(B, S, D_OUT)


# revision 32
# speedup vs baseline: 1.0496x; 1.0047x over previous
"""BitLinear (per-token int8 activation quant + ternary weight quant + matmul)
as a Bass/Tile kernel on 8 Trainium2 NeuronCores.

Strategy (data-parallel tokens + tensor-parallel weight-mean + slab rotation):
  - x [4,2048,4096] -> [8192,4096]; each core quantizes and matmuls its own
    1024-token slab against the FULL weight; outputs concatenate on tokens.
  - Each core receives W pre-rolled by 512*i rows (host-side np.roll, pure
    layout). Rows [0:512) of its (rolled) W are both its 1/8 mean-shard AND
    the weights of its first two output slabs. A 512B AllReduce combines the
    per-core |W| partial sums. The host un-rolls the output columns after.
  - The host ALSO ships x transposed (xt [4096, 1024] bf16, pure layout):
    q is generated directly in the transposed layout the PE needs, so the
    kernel needs NO DMA transposes before the AllReduce completes (in-flight
    collectives serialize all xbar transposes -- measured on hw).  Per-token
    scales reach the transposed layout via a small DRAM bounce-gather plus a
    gpsimd partition_broadcast.
  - q = rint(x*s) and tw in {-1,0,1} are exact in bf16 => the bf16 matmul
    with fp32 PSUM accumulation is EXACT integer arithmetic; per-token
    dequant scales applied on the PSUM->SBUF copy.
  - Engine split: vector does amax/q-gen/ternarize-finish/dequant; scalar
    does the W*swt+MAGIC activation and streams W; sync streams x/xt and the
    (post-AR) twT transposes; gpsimd does the AllReduce chain, broadcasts
    and output stores.  W is ternarized in [128,2048] half-blocks.
  - OF_CHUNK=256 (16 slabs), ternarize runs two slabs ahead of the matmul.
"""
import numpy as np
from ml_dtypes import bfloat16
from contextlib import ExitStack

N_CORES = 8
B, S, D_IN, D_OUT = 4, 2048, 4096, 4096
TOK = B * S                  # 8192
TOK_PC = TOK // N_CORES      # 1024 tokens per core
N_TOK_TILES = TOK_PC // 128  # 8
N_K = D_IN // 128            # 32 contraction tiles
OF_CHUNK = 256
N_SLAB = D_OUT // OF_CHUNK   # 16
SHARD_ROWS = D_OUT // N_CORES  # 512 weight rows per core for the mean
HD = D_IN // 2               # 2048 half-row
HT = TOK_PC // 2             # 512 half of the tokens
EPS = 1e-5
MAGIC = float(np.float32(1.5 * 2 ** 23))   # fp32 round-to-nearest-even trick
MEAN_SCALE = float(np.float32(1.0 / (D_IN * D_OUT)))  # 2^-24, exact
INV127 = float(np.float32(1.0 / 127.0))

_CACHE = {}


def _build_module():
    import concourse.bacc as bacc
    import concourse.tile as tile
    import concourse.mybir as mybir
    import concourse.bass_isa as bass_isa

    dt = mybir.dt
    AF = mybir.ActivationFunctionType
    AL = mybir.AluOpType
    AX = mybir.AxisListType

    nc = bacc.Bacc(
        "TRN2", target_bir_lowering=False, debug=False, num_devices=N_CORES
    )
    xs = nc.dram_tensor("xs", [TOK_PC, D_IN], dt.bfloat16, kind="ExternalInput").ap()
    xt = nc.dram_tensor("xt", [D_IN, TOK_PC], dt.bfloat16, kind="ExternalInput").ap()
    wf = nc.dram_tensor("wf", [D_OUT, D_IN], dt.float32, kind="ExternalInput").ap()
    out = nc.dram_tensor("out", [TOK_PC, D_OUT], dt.bfloat16, kind="ExternalOutput").ap()

    with tile.TileContext(nc) as tc, ExitStack() as ctx:
        stats = ctx.enter_context(tc.tile_pool(name="stats", bufs=1))
        qT_pool = ctx.enter_context(tc.tile_pool(name="qT", bufs=N_K))
        big = ctx.enter_context(tc.tile_pool(name="big", bufs=6))
        xtp = ctx.enter_context(tc.tile_pool(name="xtp", bufs=3))
        twc_pool = ctx.enter_context(tc.tile_pool(name="twcp", bufs=4))
        twTp = ctx.enter_context(tc.tile_pool(name="twT", bufs=3))
        op = ctx.enter_context(tc.tile_pool(name="op", bufs=4))
        pp = ctx.enter_context(tc.tile_pool(name="pp", bufs=7, space="PSUM"))
        dram = ctx.enter_context(tc.tile_pool(name="dram", bufs=2, space="DRAM"))

        amc = stats.tile([128, N_TOK_TILES], dt.float32, tag="amc")
        am2 = stats.tile([128, N_TOK_TILES], dt.float32, tag="am2")
        sca = stats.tile([128, N_TOK_TILES], dt.float32, tag="sca")
        dq = stats.tile([128, N_TOK_TILES], dt.float32, tag="dq")
        wme = stats.tile([128, 1], dt.float32, tag="wme")
        swt = stats.tile([128, 1], dt.float32, tag="swt")
        wp = stats.tile([128, 8], dt.float32, tag="wp")
        w16 = stats.tile([128, 16], dt.float32, tag="w16")
        zr = stats.tile([128, 1], dt.float32, tag="zr")
        gtot = stats.tile([128, 1], dt.float32, tag="gtot")
        gl = stats.tile([128, 1], dt.float32, tag="gl")
        scaT = stats.tile([1, TOK_PC], dt.float32, tag="scaT")
        scaB = stats.tile([128, TOK_PC], dt.float32, tag="scaB")
        vscr = stats.tile([128, HT], dt.float32, tag="vscr")

        arin = dram.tile([128, 1], dt.float32, tag="arin")
        arout = dram.tile([128, 1], dt.float32, tag="arout")
        scad = [
            dram.tile([128, 4], dt.float32, tag="scad", name=f"scad{h}")
            for h in range(2)
        ]

        # ---- |W| mean shard = rows [0:512) of the rolled W ----
        # blocks 0-2 stay resident as 6 half-tiles (they are also slab-0/1
        # weights); block 3 is reduced from throwaway halves and re-read.
        shard_halves = []
        with nc.named_scope("wmean"), tc.tile_pool(name="xq0", bufs=2) as xq0:
            for hh in range(6):
                blk, h = hh // 2, hh % 2
                wt = big.tile([128, HD], dt.float32, tag="big", name=f"sh{hh}")
                shard_halves.append(wt)
                nc.scalar.dma_start(
                    wt[:], wf[blk * 128:(blk + 1) * 128, h * HD:(h + 1) * HD]
                )
                nc.vector.tensor_reduce(
                    w16[:],
                    wt[:].rearrange("p (a b) -> p a b", b=128),
                    axis=AX.X, op=AL.add, apply_absolute_value=True,
                )
                nc.vector.tensor_reduce(
                    wp[:, hh:hh + 1], w16[:], axis=AX.X, op=AL.add
                )
            # block 3 of the shard: throwaway quarter tiles
            for qq in range(4):
                wt = xq0.tile([128, HD // 2], dt.float32, tag="xq", name=f"s3_{qq}")
                nc.scalar.dma_start(
                    wt[:], wf[3 * 128:4 * 128, qq * (HD // 2):(qq + 1) * (HD // 2)]
                )
                nc.vector.tensor_reduce(
                    w16[:, :8],
                    wt[:].rearrange("p (a b) -> p a b", b=128),
                    axis=AX.X, op=AL.add, apply_absolute_value=True,
                )
                nc.vector.tensor_reduce(
                    wp[:, 6 + (qq // 2):7 + (qq // 2)] if qq % 2 == 0 else
                    w16[:, 8:9],
                    w16[:, :8], axis=AX.X, op=AL.add
                )
                if qq % 2 == 1:
                    nc.vector.tensor_tensor(
                        wp[:, 6 + (qq // 2):7 + (qq // 2)],
                        wp[:, 6 + (qq // 2):7 + (qq // 2)],
                        w16[:, 8:9], op=AL.add,
                    )
            nc.vector.tensor_reduce(zr[:], wp[:], axis=AX.X, op=AL.add)
            nc.gpsimd.partition_all_reduce(
                gtot[:], zr[:], channels=128, reduce_op=bass_isa.ReduceOp.add
            )
            nc.gpsimd.dma_start(arin[:], gtot[:])
            nc.gpsimd.collective_compute(
                "AllReduce",
                mybir.AluOpType.add,
                replica_groups=[list(range(N_CORES))],
                ins=[arin.opt()],
                outs=[arout.opt()],
            )

        # ---- x amax (row-major x) -> per-token scales (AR-independent) ----
        with nc.named_scope("xquant"), tc.tile_pool(name="xq", bufs=3) as xq:
            for t in range(N_TOK_TILES):
                for h in range(2):
                    xth = xq.tile([128, HD], dt.bfloat16, tag="xq", name=f"xt{t}_{h}")
                    nc.sync.dma_start(
                        xth[:], xs[t * 128:(t + 1) * 128, h * HD:(h + 1) * HD]
                    )
                    nc.vector.tensor_reduce(
                        (amc if h == 0 else am2)[:, t:t + 1],
                        xth[:], axis=AX.X, op=AL.max, apply_absolute_value=True,
                    )
                # amax = max(half0, half1, EPS); s = 127/amax
                nc.vector.tensor_tensor(
                    amc[:, t:t + 1], amc[:, t:t + 1], am2[:, t:t + 1], op=AL.max
                )
                nc.vector.tensor_scalar(
                    amc[:, t:t + 1], amc[:, t:t + 1], EPS, None, op0=AL.max
                )
                nc.vector.reciprocal(sca[:, t:t + 1], amc[:, t:t + 1])
                nc.vector.tensor_scalar(
                    sca[:, t:t + 1], sca[:, t:t + 1], 127.0, None, op0=AL.mult
                )
                # after each half of the token tiles: bounce the scales (as
                # bf16) through DRAM into token-major [1, 512] and broadcast
                # to all partitions (plain DMAs only -- no xbar transpose).
                if t == 3 or t == 7:
                    hb = t // 4
                    nc.scalar.dma_start(
                        scad[hb][:], sca[:, hb * 4:(hb + 1) * 4]
                    )
                    nc.scalar.dma_start(
                        scaT[0:1, hb * HT:(hb + 1) * HT],
                        scad[hb][:].rearrange("p c -> c p"),
                    )
                    nc.gpsimd.partition_broadcast(
                        scaB[:, hb * HT:(hb + 1) * HT],
                        scaT[0:1, hb * HT:(hb + 1) * HT],
                    )

            # ---- q-gen directly in transposed layout (bf16 throughout) ----
            qT_tiles = [
                qT_pool.tile([128, TOK_PC], dt.bfloat16, tag="qT", name=f"qT{k}")
                for k in range(N_K)
            ]
            for k in range(N_K):
                xtk = xtp.tile([128, TOK_PC], dt.bfloat16, tag="xt", name=f"x{k}")
                nc.sync.dma_start(xtk[:], xt[k * 128:(k + 1) * 128, :])
                for hb in range(2):
                    cs = slice(hb * HT, (hb + 1) * HT)
                    nc.vector.tensor_tensor(
                        vscr[:], xtk[:, cs], scaB[:, cs], op=AL.mult
                    )
                    nc.vector.tensor_scalar(
                        qT_tiles[k][:, cs], vscr[:], MAGIC, MAGIC,
                        op0=AL.add, op1=AL.subtract,
                    )

        # ---- AR-dependent epilogue of the mean (vector) ----
        with nc.named_scope("wmean"):
            nc.gpsimd.dma_start(gl[:], arout[:])
            nc.vector.tensor_scalar(
                wme[:], gl[:], MEAN_SCALE, EPS, op0=AL.mult, op1=AL.max
            )
            nc.vector.reciprocal(swt[:], wme[:])
            # per-token dequant scale: amax * mean|W| / 127
            nc.vector.tensor_scalar(
                dq[:], amc[:], wme[:, 0:1], INV127, op0=AL.mult, op1=AL.mult
            )

        # ---- per-slab: ternarize ~two slabs ahead, then matmul ----
        def stage_tern(c):
            twT_c = twTp.tile(
                [128, N_K, OF_CHUNK], dt.bfloat16, tag="twT", name=f"twT{c}"
            )
            for j in range(2):
                blk = 2 * c + j
                for h in range(2):
                    if blk < 3:
                        wh = shard_halves[blk * 2 + h]
                    else:
                        wh = big.tile(
                            [128, HD], dt.float32, tag="big", name=f"wt{blk}_{h}"
                        )
                        nc.scalar.dma_start(
                            wh[:],
                            wf[blk * 128:(blk + 1) * 128, h * HD:(h + 1) * HD],
                        )
                    # v = W*swt + MAGIC  (exact rint encoding in the mantissa)
                    nc.scalar.activation(
                        wh[:], wh[:], AF.Copy, bias=MAGIC, scale=swt[:, 0:1]
                    )
                    tch = twc_pool.tile(
                        [128, HD], dt.bfloat16, tag="twc", name=f"twc{blk}_{h}"
                    )
                    # rint(W*swt) to bf16 (exact, values are small ints)...
                    nc.vector.tensor_scalar(
                        tch[:], wh[:], MAGIC, None, op0=AL.subtract
                    )
                    # ...then ternary clip on bf16 (2x DVE rate)
                    nc.vector.tensor_scalar(
                        tch[:], tch[:], 1.0, -1.0, op0=AL.min, op1=AL.max
                    )
                    nc.sync.dma_start(
                        twT_c[:, h * (N_K // 2):(h + 1) * (N_K // 2),
                              j * 128:(j + 1) * 128],
                        tch[:], transpose=True,
                    )
            return twT_c

        def stage_mm(c, twT_c):
            for t in range(N_TOK_TILES):
                ps = pp.tile([128, OF_CHUNK], dt.float32, tag="ps", name=f"ps{c}_{t}")
                for k in range(N_K):
                    nc.tensor.matmul(
                        ps[:], qT_tiles[k][:, t * 128:(t + 1) * 128],
                        twT_c[:, k, :],
                        start=(k == 0), stop=(k == N_K - 1),
                    )
                ot = op.tile([128, OF_CHUNK], dt.bfloat16, tag="ot", name=f"ot{c}_{t}")
                nc.vector.tensor_scalar(
                    ot[:], ps[:], dq[:, t:t + 1], None, op0=AL.mult
                )
                nc.gpsimd.dma_start(
                    out[t * 128:(t + 1) * 128, c * OF_CHUNK:(c + 1) * OF_CHUNK],
                    ot[:],
                )

        with nc.named_scope("mm"):
            pending = [stage_tern(0), stage_tern(1)]
            for c in range(N_SLAB):
                stage_mm(c, pending.pop(0))
                if c + 2 < N_SLAB:
                    pending.append(stage_tern(c + 2))

    nc.compile()
    return nc


def _get_module():
    if "nc" not in _CACHE:
        _CACHE["nc"] = _build_module()
    return _CACHE["nc"]


def _make_in_maps(x2, w2):
    # core i gets W rolled so its mean-shard == its first two slabs' rows;
    # x is shipped both row-major (amax) and transposed (q-gen), bf16.
    maps = []
    for i in range(N_CORES):
        xsl = x2[i * TOK_PC:(i + 1) * TOK_PC].astype(bfloat16)
        maps.append({
            "xs": xsl,
            "xt": np.ascontiguousarray(xsl.T),
            "wf": np.ascontiguousarray(
                np.roll(w2, -SHARD_ROWS * i, axis=0)
            ) if i else w2,
        })
    return maps


def kernel(x: np.ndarray, weight: np.ndarray) -> np.ndarray:
    from concourse.bass_utils import run_bass_kernel_spmd

    x = np.asarray(x, dtype=np.float32)
    weight = np.asarray(weight, dtype=np.float32)
    x2 = np.ascontiguousarray(x.reshape(TOK, D_IN))
    w2 = np.ascontiguousarray(weight)

    in_maps = _make_in_maps(x2, w2)
    nc = _get_module()
    res = run_bass_kernel_spmd(nc, in_maps, list(range(N_CORES)))
    # core i's output columns are rolled by -512*i (it computed the rolled
    # weight rows in order); roll them back before concatenating tokens
    parts = [
        np.roll(np.asarray(res.results[i]["out"], dtype=np.float32),
                SHARD_ROWS * i, axis=1) if i
        else np.asarray(res.results[i]["out"], dtype=np.float32)
        for i in range(N_CORES)
    ]
    out = np.concatenate(parts, axis=0)
    return out.reshape(B, S, D_OUT)


# revision 36
# speedup vs baseline: 1.1055x; 1.0533x over previous
"""BitLinear (per-token int8 activation quant + ternary weight quant + matmul)
as a Bass/Tile kernel on 8 Trainium2 NeuronCores.

Strategy (data-parallel tokens + tensor-parallel weight-mean + slab rotation):
  - x [4,2048,4096] -> [8192,4096]; each core quantizes and matmuls its own
    1024-token slab against the FULL weight; outputs concatenate on tokens.
  - The host ships W TRANSPOSED (wft [in, out] fp32) and column-rolled by
    512*i per core (pure layout): columns [0:512) of core i's wft are both
    its 1/8 |W|-mean shard AND its first two output slabs.  A 512B AllReduce
    combines the per-core mean partials; the host un-rolls output columns.
  - The host also ships x both row-major (for the per-token amax) and
    transposed (xt [4096,1024] bf16): q is generated directly in the
    [contraction, token] layout the PE needs.  The kernel therefore needs
    ZERO on-device DMA transposes -- important because in-flight collectives
    serialize all xbar transposes (measured on hw), and because a second
    transpose ring races the first through a shared bounce region.
  - Per-token scales reach the transposed layout via a small DRAM
    bounce-gather plus a gpsimd partition_broadcast (plain DMAs only).
  - q = rint(x*s) and tw in {-1,0,1} are exact in bf16 => the bf16 matmul
    with fp32 PSUM accumulation is EXACT integer arithmetic; per-token
    dequant scales applied on the PSUM->SBUF copy.
  - Engine split: vector does amax/q-gen/ternarize-finish/dequant; scalar
    streams W and runs the W*swt+MAGIC activation; sync streams x/xt;
    gpsimd does the AllReduce chain, broadcasts and output stores.
  - OF_CHUNK=256 (16 slabs), ternarize runs two slabs ahead of the matmul,
    W streamed as [128,256] fp32 tiles (batched loads, then ACT+clip).
"""
import numpy as np
from ml_dtypes import bfloat16
from contextlib import ExitStack

N_CORES = 8
B, S, D_IN, D_OUT = 4, 2048, 4096, 4096
TOK = B * S                  # 8192
TOK_PC = TOK // N_CORES      # 1024 tokens per core
N_TOK_TILES = TOK_PC // 128  # 8
N_K = D_IN // 128            # 32 contraction tiles
OF_CHUNK = 256
N_SLAB = D_OUT // OF_CHUNK   # 16
SHARD_ROWS = D_OUT // N_CORES  # 512 weight rows per core for the mean
HD = D_IN // 2               # 2048 half-row
HT = TOK_PC // 2             # 512 half of the tokens
EPS = 1e-5
MAGIC = float(np.float32(1.5 * 2 ** 23))   # fp32 round-to-nearest-even trick
MEAN_SCALE = float(np.float32(1.0 / (D_IN * D_OUT)))  # 2^-24, exact
INV127 = float(np.float32(1.0 / 127.0))

_CACHE = {}


def _build_module():
    import concourse.bacc as bacc
    import concourse.tile as tile
    import concourse.mybir as mybir
    import concourse.bass_isa as bass_isa

    dt = mybir.dt
    AF = mybir.ActivationFunctionType
    AL = mybir.AluOpType
    AX = mybir.AxisListType

    nc = bacc.Bacc(
        "TRN2", target_bir_lowering=False, debug=False, num_devices=N_CORES
    )
    xs = nc.dram_tensor("xs", [TOK_PC, D_IN], dt.bfloat16, kind="ExternalInput").ap()
    xt = nc.dram_tensor("xt", [D_IN, TOK_PC], dt.bfloat16, kind="ExternalInput").ap()
    wft = nc.dram_tensor("wft", [D_IN, D_OUT], dt.float32, kind="ExternalInput").ap()
    out = nc.dram_tensor("out", [TOK_PC, D_OUT], dt.bfloat16, kind="ExternalOutput").ap()

    with tile.TileContext(nc) as tc, ExitStack() as ctx:
        stats = ctx.enter_context(tc.tile_pool(name="stats", bufs=1))
        qT_pool = ctx.enter_context(tc.tile_pool(name="qT", bufs=N_K))
        big = ctx.enter_context(tc.tile_pool(name="big", bufs=48))
        xtp = ctx.enter_context(tc.tile_pool(name="xtp", bufs=4))
        twTp = ctx.enter_context(tc.tile_pool(name="twT", bufs=3))
        op = ctx.enter_context(tc.tile_pool(name="op", bufs=4))
        pp = ctx.enter_context(tc.tile_pool(name="pp", bufs=7, space="PSUM"))
        dram = ctx.enter_context(tc.tile_pool(name="dram", bufs=2, space="DRAM"))

        amc = stats.tile([128, N_TOK_TILES], dt.float32, tag="amc")
        am2 = stats.tile([128, N_TOK_TILES], dt.float32, tag="am2")
        sca = stats.tile([128, N_TOK_TILES], dt.float32, tag="sca")
        dq = stats.tile([128, N_TOK_TILES], dt.float32, tag="dq")
        wme = stats.tile([128, 1], dt.float32, tag="wme")
        swt = stats.tile([128, 1], dt.float32, tag="swt")
        wA = stats.tile([128, N_K], dt.float32, tag="wA")
        zr = stats.tile([128, 1], dt.float32, tag="zr")
        gtot = stats.tile([128, 1], dt.float32, tag="gtot")
        gl = stats.tile([128, 1], dt.float32, tag="gl")
        scaT = stats.tile([1, TOK_PC], dt.float32, tag="scaT")
        scaB = stats.tile([128, TOK_PC], dt.float32, tag="scaB")
        vscr = stats.tile([128, HT], dt.float32, tag="vscr")

        arin = dram.tile([128, 1], dt.float32, tag="arin")
        arout = dram.tile([128, 1], dt.float32, tag="arout")
        scad = [
            dram.tile([128, 4], dt.float32, tag="scad", name=f"scad{h}")
            for h in range(2)
        ]

        # ---- |W| mean shard = rows [0:512) of the rolled W ----
        # blocks 0-2 stay resident as 6 half-tiles (they are also slab-0/1
        # weights); block 3 is reduced from throwaway halves and re-read.
        with nc.named_scope("wmean"), tc.tile_pool(name="shm", bufs=4) as shm:
            for k in range(N_K):
                wt = shm.tile([128, 2 * OF_CHUNK], dt.float32, tag="shm",
                              name=f"sh{k}")
                nc.scalar.dma_start(
                    wt[:], wft[k * 128:(k + 1) * 128, 0:2 * OF_CHUNK]
                )
                nc.vector.tensor_reduce(
                    wA[:, k:k + 1], wt[:], axis=AX.X, op=AL.add,
                    apply_absolute_value=True,
                )
            nc.vector.tensor_reduce(zr[:], wA[:], axis=AX.X, op=AL.add)
            nc.gpsimd.partition_all_reduce(
                gtot[:], zr[:], channels=128, reduce_op=bass_isa.ReduceOp.add
            )
            nc.gpsimd.dma_start(arin[:], gtot[:])
            nc.gpsimd.collective_compute(
                "AllReduce",
                mybir.AluOpType.add,
                replica_groups=[list(range(N_CORES))],
                ins=[arin.opt()],
                outs=[arout.opt()],
            )

        # ---- x amax (row-major x) -> per-token scales (AR-independent) ----
        with nc.named_scope("xquant"), tc.tile_pool(name="xq", bufs=3) as xq:
            for t in range(N_TOK_TILES):
                for h in range(2):
                    xth = xq.tile([128, HD], dt.bfloat16, tag="xq", name=f"xt{t}_{h}")
                    nc.sync.dma_start(
                        xth[:], xs[t * 128:(t + 1) * 128, h * HD:(h + 1) * HD]
                    )
                    nc.vector.tensor_reduce(
                        (amc if h == 0 else am2)[:, t:t + 1],
                        xth[:], axis=AX.X, op=AL.max, apply_absolute_value=True,
                    )
                # amax = max(half0, half1, EPS); s = 127/amax
                nc.vector.tensor_tensor(
                    amc[:, t:t + 1], amc[:, t:t + 1], am2[:, t:t + 1], op=AL.max
                )
                nc.vector.tensor_scalar(
                    amc[:, t:t + 1], amc[:, t:t + 1], EPS, None, op0=AL.max
                )
                nc.vector.reciprocal(sca[:, t:t + 1], amc[:, t:t + 1])
                nc.vector.tensor_scalar(
                    sca[:, t:t + 1], sca[:, t:t + 1], 127.0, None, op0=AL.mult
                )
                # after each half of the token tiles: bounce the scales (as
                # bf16) through DRAM into token-major [1, 512] and broadcast
                # to all partitions (plain DMAs only -- no xbar transpose).
                if t == 3 or t == 7:
                    hb = t // 4
                    nc.scalar.dma_start(
                        scad[hb][:], sca[:, hb * 4:(hb + 1) * 4]
                    )
                    nc.scalar.dma_start(
                        scaT[0:1, hb * HT:(hb + 1) * HT],
                        scad[hb][:].rearrange("p c -> c p"),
                    )
                    nc.gpsimd.partition_broadcast(
                        scaB[:, hb * HT:(hb + 1) * HT],
                        scaT[0:1, hb * HT:(hb + 1) * HT],
                    )

            # ---- q-gen directly in transposed layout (bf16 throughout) ----
            qT_tiles = [
                qT_pool.tile([128, TOK_PC], dt.bfloat16, tag="qT", name=f"qT{k}")
                for k in range(N_K)
            ]
            for k in range(N_K):
                xtk = xtp.tile([128, TOK_PC], dt.bfloat16, tag="xt", name=f"x{k}")
                nc.sync.dma_start(xtk[:], xt[k * 128:(k + 1) * 128, :])
                for hb in range(2):
                    cs = slice(hb * HT, (hb + 1) * HT)
                    nc.vector.tensor_tensor(
                        vscr[:], xtk[:, cs], scaB[:, cs], op=AL.mult
                    )
                    nc.vector.tensor_scalar(
                        qT_tiles[k][:, cs], vscr[:], MAGIC, MAGIC,
                        op0=AL.add, op1=AL.subtract,
                    )

        # ---- AR-dependent epilogue of the mean (vector) ----
        with nc.named_scope("wmean"):
            nc.gpsimd.dma_start(gl[:], arout[:])
            nc.vector.tensor_scalar(
                wme[:], gl[:], MEAN_SCALE, EPS, op0=AL.mult, op1=AL.max
            )
            nc.vector.reciprocal(swt[:], wme[:])
            # per-token dequant scale: amax * mean|W| / 127
            nc.vector.tensor_scalar(
                dq[:], amc[:], wme[:, 0:1], INV127, op0=AL.mult, op1=AL.mult
            )

        # ---- per-slab: ternarize ~two slabs ahead, then matmul ----
        def stage_tern(c):
            twT_c = twTp.tile(
                [128, N_K, OF_CHUNK], dt.bfloat16, tag="twT", name=f"twT{c}"
            )
            whs = []
            for k in range(N_K):
                wt = big.tile(
                    [128, OF_CHUNK], dt.float32, tag="big", name=f"w{c}_{k}"
                )
                nc.scalar.dma_start(
                    wt[:],
                    wft[k * 128:(k + 1) * 128, c * OF_CHUNK:(c + 1) * OF_CHUNK],
                )
                whs.append(wt)
            for k in range(N_K):
                # v = W*swt + MAGIC  (exact rint encoding in the mantissa)
                nc.scalar.activation(
                    whs[k][:], whs[k][:], AF.Copy, bias=MAGIC, scale=swt[:, 0:1]
                )
                # rint to bf16 (exact, small ints), then ternary clip
                nc.vector.tensor_scalar(
                    twT_c[:, k, :], whs[k][:], MAGIC, None, op0=AL.subtract
                )
                nc.vector.tensor_scalar(
                    twT_c[:, k, :], twT_c[:, k, :], 1.0, -1.0,
                    op0=AL.min, op1=AL.max,
                )
            return twT_c

        def stage_mm(c, twT_c):
            for t in range(N_TOK_TILES):
                ps = pp.tile([128, OF_CHUNK], dt.float32, tag="ps", name=f"ps{c}_{t}")
                for k in range(N_K):
                    nc.tensor.matmul(
                        ps[:], qT_tiles[k][:, t * 128:(t + 1) * 128],
                        twT_c[:, k, :],
                        start=(k == 0), stop=(k == N_K - 1),
                    )
                ot = op.tile([128, OF_CHUNK], dt.bfloat16, tag="ot", name=f"ot{c}_{t}")
                nc.vector.tensor_scalar(
                    ot[:], ps[:], dq[:, t:t + 1], None, op0=AL.mult
                )
                nc.gpsimd.dma_start(
                    out[t * 128:(t + 1) * 128, c * OF_CHUNK:(c + 1) * OF_CHUNK],
                    ot[:],
                )

        with nc.named_scope("mm"):
            pending = [stage_tern(0), stage_tern(1)]
            for c in range(N_SLAB):
                stage_mm(c, pending.pop(0))
                if c + 2 < N_SLAB:
                    pending.append(stage_tern(c + 2))

    nc.compile()
    return nc


def _get_module():
    if "nc" not in _CACHE:
        _CACHE["nc"] = _build_module()
    return _CACHE["nc"]


def _make_in_maps(x2, w2):
    # core i gets W transposed (host layout prep) and column-rolled so its
    # mean-shard == its first two slabs' columns; x is shipped both
    # row-major (amax) and transposed (q-gen), bf16.
    wT = np.ascontiguousarray(w2.T)
    maps = []
    for i in range(N_CORES):
        xsl = x2[i * TOK_PC:(i + 1) * TOK_PC].astype(bfloat16)
        maps.append({
            "xs": xsl,
            "xt": np.ascontiguousarray(xsl.T),
            "wft": np.ascontiguousarray(
                np.roll(wT, -SHARD_ROWS * i, axis=1)
            ) if i else wT,
        })
    return maps


def kernel(x: np.ndarray, weight: np.ndarray) -> np.ndarray:
    from concourse.bass_utils import run_bass_kernel_spmd

    x = np.asarray(x, dtype=np.float32)
    weight = np.asarray(weight, dtype=np.float32)
    x2 = np.ascontiguousarray(x.reshape(TOK, D_IN))
    w2 = np.ascontiguousarray(weight)

    in_maps = _make_in_maps(x2, w2)
    nc = _get_module()
    res = run_bass_kernel_spmd(nc, in_maps, list(range(N_CORES)))
    # core i's output columns are rolled by -512*i (it computed the rolled
    # weight rows in order); roll them back before concatenating tokens
    parts = [
        np.roll(np.asarray(res.results[i]["out"], dtype=np.float32),
                SHARD_ROWS * i, axis=1) if i
        else np.asarray(res.results[i]["out"], dtype=np.float32)
        for i in range(N_CORES)
    ]
    out = np.concatenate(parts, axis=0)
    return out.reshape(B, S, D_OUT)


# revision 37
# speedup vs baseline: 1.1540x; 1.0439x over previous
"""BitLinear (per-token int8 activation quant + ternary weight quant + matmul)
as a Bass/Tile kernel on 8 Trainium2 NeuronCores.

Strategy (data-parallel tokens + tensor-parallel weight-mean + slab rotation):
  - x [4,2048,4096] -> [8192,4096]; each core quantizes and matmuls its own
    1024-token slab against the FULL weight; outputs concatenate on tokens.
  - The host ships W TRANSPOSED (wft [in, out] fp32) and column-rolled by
    512*i per core (pure layout): columns [0:512) of core i's wft are both
    its 1/8 |W|-mean shard AND its first two output slabs.  A 512B AllReduce
    combines the per-core mean partials; the host un-rolls output columns.
  - The host also ships x both row-major (for the per-token amax) and
    transposed (xt [4096,1024] bf16): q is generated directly in the
    [contraction, token] layout the PE needs.  The kernel therefore needs
    ZERO on-device DMA transposes -- important because in-flight collectives
    serialize all xbar transposes (measured on hw), and because a second
    transpose ring races the first through a shared bounce region.
  - Per-token scales reach the transposed layout via a small DRAM
    bounce-gather plus a gpsimd partition_broadcast (plain DMAs only).
  - q = rint(x*s) and tw in {-1,0,1} are exact in bf16 => the bf16 matmul
    with fp32 PSUM accumulation is EXACT integer arithmetic; per-token
    dequant scales applied on the PSUM->SBUF copy.
  - Engine split: vector does amax/q-gen/ternarize-finish/dequant; scalar
    streams W and runs the W*swt+MAGIC activation; sync streams x/xt;
    gpsimd does the AllReduce chain, broadcasts and output stores.
  - OF_CHUNK=256 (16 slabs), ternarize runs two slabs ahead of the matmul,
    W streamed as [128,256] fp32 tiles (batched loads, then ACT+clip).
"""
import numpy as np
from ml_dtypes import bfloat16
from contextlib import ExitStack

N_CORES = 8
B, S, D_IN, D_OUT = 4, 2048, 4096, 4096
TOK = B * S                  # 8192
TOK_PC = TOK // N_CORES      # 1024 tokens per core
N_TOK_TILES = TOK_PC // 128  # 8
N_K = D_IN // 128            # 32 contraction tiles
OF_CHUNK = 256
N_SLAB = D_OUT // OF_CHUNK   # 16
SHARD_ROWS = D_OUT // N_CORES  # 512 weight rows per core for the mean
HD = D_IN // 2               # 2048 half-row
HT = TOK_PC // 2             # 512 half of the tokens
EPS = 1e-5
MAGIC = float(np.float32(1.5 * 2 ** 23))   # fp32 round-to-nearest-even trick
MEAN_SCALE = float(np.float32(1.0 / (D_IN * D_OUT)))  # 2^-24, exact
INV127 = float(np.float32(1.0 / 127.0))

_CACHE = {}


def _build_module():
    import concourse.bacc as bacc
    import concourse.tile as tile
    import concourse.mybir as mybir
    import concourse.bass_isa as bass_isa

    dt = mybir.dt
    AF = mybir.ActivationFunctionType
    AL = mybir.AluOpType
    AX = mybir.AxisListType

    nc = bacc.Bacc(
        "TRN2", target_bir_lowering=False, debug=False, num_devices=N_CORES
    )
    xs = nc.dram_tensor("xs", [TOK_PC, D_IN], dt.bfloat16, kind="ExternalInput").ap()
    xt = nc.dram_tensor("xt", [D_IN, TOK_PC], dt.bfloat16, kind="ExternalInput").ap()
    wft = nc.dram_tensor("wft", [D_IN, D_OUT], dt.float32, kind="ExternalInput").ap()
    out = nc.dram_tensor("out", [TOK_PC, D_OUT], dt.bfloat16, kind="ExternalOutput").ap()

    with tile.TileContext(nc) as tc, ExitStack() as ctx:
        stats = ctx.enter_context(tc.tile_pool(name="stats", bufs=1))
        qT_pool = ctx.enter_context(tc.tile_pool(name="qT", bufs=N_K))
        big = ctx.enter_context(tc.tile_pool(name="big", bufs=48))
        xtp = ctx.enter_context(tc.tile_pool(name="xtp", bufs=4))
        twTp = ctx.enter_context(tc.tile_pool(name="twT", bufs=3))
        op = ctx.enter_context(tc.tile_pool(name="op", bufs=4))
        pp = ctx.enter_context(tc.tile_pool(name="pp", bufs=7, space="PSUM"))
        dram = ctx.enter_context(tc.tile_pool(name="dram", bufs=2, space="DRAM"))

        amc = stats.tile([128, N_TOK_TILES], dt.float32, tag="amc")
        am2 = stats.tile([128, N_TOK_TILES], dt.float32, tag="am2")
        sca = stats.tile([128, N_TOK_TILES], dt.float32, tag="sca")
        dq = stats.tile([128, N_TOK_TILES], dt.float32, tag="dq")
        wme = stats.tile([128, 1], dt.float32, tag="wme")
        swt = stats.tile([128, 1], dt.float32, tag="swt")
        wA = stats.tile([128, N_K], dt.float32, tag="wA")
        zr = stats.tile([128, 1], dt.float32, tag="zr")
        gtot = stats.tile([128, 1], dt.float32, tag="gtot")
        gl = stats.tile([128, 1], dt.float32, tag="gl")
        scaT = stats.tile([1, TOK_PC], dt.float32, tag="scaT")
        scaB = stats.tile([128, TOK_PC], dt.float32, tag="scaB")
        vscr = stats.tile([128, HT], dt.float32, tag="vscr")

        arin = dram.tile([128, 1], dt.float32, tag="arin")
        arout = dram.tile([128, 1], dt.float32, tag="arout")
        scad = [
            dram.tile([128, 4], dt.float32, tag="scad", name=f"scad{h}")
            for h in range(2)
        ]

        # ---- |W| mean shard = rows [0:512) of the rolled W ----
        # blocks 0-2 stay resident as 6 half-tiles (they are also slab-0/1
        # weights); block 3 is reduced from throwaway halves and re-read.
        with nc.named_scope("wmean"), tc.tile_pool(name="shm", bufs=4) as shm:
            for k in range(N_K):
                wt = shm.tile([128, 2 * OF_CHUNK], dt.float32, tag="shm",
                              name=f"sh{k}")
                nc.scalar.dma_start(
                    wt[:], wft[k * 128:(k + 1) * 128, 0:2 * OF_CHUNK]
                )
                nc.vector.tensor_reduce(
                    wA[:, k:k + 1], wt[:], axis=AX.X, op=AL.add,
                    apply_absolute_value=True,
                )
            nc.vector.tensor_reduce(zr[:], wA[:], axis=AX.X, op=AL.add)
            nc.gpsimd.partition_all_reduce(
                gtot[:], zr[:], channels=128, reduce_op=bass_isa.ReduceOp.add
            )
            nc.gpsimd.dma_start(arin[:], gtot[:])
            nc.gpsimd.collective_compute(
                "AllReduce",
                mybir.AluOpType.add,
                replica_groups=[list(range(N_CORES))],
                ins=[arin.opt()],
                outs=[arout.opt()],
            )

        # ---- x amax (row-major x) -> per-token scales (AR-independent) ----
        with nc.named_scope("xquant"), tc.tile_pool(name="xq", bufs=3) as xq:
            for t in range(N_TOK_TILES):
                for h in range(2):
                    xth = xq.tile([128, HD], dt.bfloat16, tag="xq", name=f"xt{t}_{h}")
                    nc.sync.dma_start(
                        xth[:], xs[t * 128:(t + 1) * 128, h * HD:(h + 1) * HD]
                    )
                    nc.vector.tensor_reduce(
                        (amc if h == 0 else am2)[:, t:t + 1],
                        xth[:], axis=AX.X, op=AL.max, apply_absolute_value=True,
                    )
                # amax = max(half0, half1, EPS); s = 127/amax
                nc.vector.tensor_tensor(
                    amc[:, t:t + 1], amc[:, t:t + 1], am2[:, t:t + 1], op=AL.max
                )
                nc.vector.tensor_scalar(
                    amc[:, t:t + 1], amc[:, t:t + 1], EPS, None, op0=AL.max
                )
                nc.vector.reciprocal(sca[:, t:t + 1], amc[:, t:t + 1])
                nc.vector.tensor_scalar(
                    sca[:, t:t + 1], sca[:, t:t + 1], 127.0, None, op0=AL.mult
                )
                # after each half of the token tiles: bounce the scales (as
                # bf16) through DRAM into token-major [1, 512] and broadcast
                # to all partitions (plain DMAs only -- no xbar transpose).
                if t == 3 or t == 7:
                    hb = t // 4
                    nc.scalar.dma_start(
                        scad[hb][:], sca[:, hb * 4:(hb + 1) * 4]
                    )
                    nc.scalar.dma_start(
                        scaT[0:1, hb * HT:(hb + 1) * HT],
                        scad[hb][:].rearrange("p c -> c p"),
                    )
                    nc.gpsimd.partition_broadcast(
                        scaB[:, hb * HT:(hb + 1) * HT],
                        scaT[0:1, hb * HT:(hb + 1) * HT],
                    )

            # ---- q-gen directly in transposed layout (bf16 throughout) ----
            qT_tiles = [
                qT_pool.tile([128, TOK_PC], dt.bfloat16, tag="qT", name=f"qT{k}")
                for k in range(N_K)
            ]
            for k in range(N_K):
                xtk = xtp.tile([128, TOK_PC], dt.bfloat16, tag="xt", name=f"x{k}")
                nc.sync.dma_start(xtk[:], xt[k * 128:(k + 1) * 128, :])
                for hb in range(2):
                    cs = slice(hb * HT, (hb + 1) * HT)
                    nc.vector.tensor_tensor(
                        vscr[:], xtk[:, cs], scaB[:, cs], op=AL.mult
                    )
                    nc.vector.tensor_scalar(
                        qT_tiles[k][:, cs], vscr[:], MAGIC, MAGIC,
                        op0=AL.add, op1=AL.subtract,
                    )

        # ---- AR-dependent epilogue of the mean (vector) ----
        with nc.named_scope("wmean"):
            nc.gpsimd.dma_start(gl[:], arout[:])
            nc.vector.tensor_scalar(
                wme[:], gl[:], MEAN_SCALE, EPS, op0=AL.mult, op1=AL.max
            )
            nc.vector.reciprocal(swt[:], wme[:])
            # per-token dequant scale: amax * mean|W| / 127
            nc.vector.tensor_scalar(
                dq[:], amc[:], wme[:, 0:1], INV127, op0=AL.mult, op1=AL.mult
            )

        # ---- per-slab: ternarize ~two slabs ahead, then matmul ----
        def stage_tern(c):
            twT_c = twTp.tile(
                [128, N_K, OF_CHUNK], dt.bfloat16, tag="twT", name=f"twT{c}"
            )
            whs = []
            for k in range(N_K):
                wt = big.tile(
                    [128, OF_CHUNK], dt.float32, tag="big", name=f"w{c}_{k}"
                )
                nc.scalar.dma_start(
                    wt[:],
                    wft[k * 128:(k + 1) * 128, c * OF_CHUNK:(c + 1) * OF_CHUNK],
                )
                whs.append(wt)
            for k in range(N_K):
                # v = W*swt + MAGIC  (exact rint encoding in the mantissa);
                # alternate engines -- scalar was the steady-state pacer
                if k % 2 == 0:
                    nc.scalar.activation(
                        whs[k][:], whs[k][:], AF.Copy,
                        bias=MAGIC, scale=swt[:, 0:1],
                    )
                else:
                    nc.vector.tensor_scalar(
                        whs[k][:], whs[k][:], swt[:, 0:1], MAGIC,
                        op0=AL.mult, op1=AL.add,
                    )
                # rint to bf16 (exact, small ints), then ternary clip
                nc.vector.tensor_scalar(
                    twT_c[:, k, :], whs[k][:], MAGIC, None, op0=AL.subtract
                )
                nc.vector.tensor_scalar(
                    twT_c[:, k, :], twT_c[:, k, :], 1.0, -1.0,
                    op0=AL.min, op1=AL.max,
                )
            return twT_c

        def stage_mm(c, twT_c):
            for t in range(N_TOK_TILES):
                ps = pp.tile([128, OF_CHUNK], dt.float32, tag="ps", name=f"ps{c}_{t}")
                for k in range(N_K):
                    nc.tensor.matmul(
                        ps[:], qT_tiles[k][:, t * 128:(t + 1) * 128],
                        twT_c[:, k, :],
                        start=(k == 0), stop=(k == N_K - 1),
                    )
                ot = op.tile([128, OF_CHUNK], dt.bfloat16, tag="ot", name=f"ot{c}_{t}")
                nc.vector.tensor_scalar(
                    ot[:], ps[:], dq[:, t:t + 1], None, op0=AL.mult
                )
                nc.gpsimd.dma_start(
                    out[t * 128:(t + 1) * 128, c * OF_CHUNK:(c + 1) * OF_CHUNK],
                    ot[:],
                )

        with nc.named_scope("mm"):
            pending = [stage_tern(0), stage_tern(1)]
            for c in range(N_SLAB):
                stage_mm(c, pending.pop(0))
                if c + 2 < N_SLAB:
                    pending.append(stage_tern(c + 2))

    nc.compile()
    return nc


def _get_module():
    if "nc" not in _CACHE:
        _CACHE["nc"] = _build_module()
    return _CACHE["nc"]


def _make_in_maps(x2, w2):
    # core i gets W transposed (host layout prep) and column-rolled so its
    # mean-shard == its first two slabs' columns; x is shipped both
    # row-major (amax) and transposed (q-gen), bf16.
    wT = np.ascontiguousarray(w2.T)
    maps = []
    for i in range(N_CORES):
        xsl = x2[i * TOK_PC:(i + 1) * TOK_PC].astype(bfloat16)
        maps.append({
            "xs": xsl,
            "xt": np.ascontiguousarray(xsl.T),
            "wft": np.ascontiguousarray(
                np.roll(wT, -SHARD_ROWS * i, axis=1)
            ) if i else wT,
        })
    return maps


def kernel(x: np.ndarray, weight: np.ndarray) -> np.ndarray:
    from concourse.bass_utils import run_bass_kernel_spmd

    x = np.asarray(x, dtype=np.float32)
    weight = np.asarray(weight, dtype=np.float32)
    x2 = np.ascontiguousarray(x.reshape(TOK, D_IN))
    w2 = np.ascontiguousarray(weight)

    in_maps = _make_in_maps(x2, w2)
    nc = _get_module()
    res = run_bass_kernel_spmd(nc, in_maps, list(range(N_CORES)))
    # core i's output columns are rolled by -512*i (it computed the rolled
    # weight rows in order); roll them back before concatenating tokens
    parts = [
        np.roll(np.asarray(res.results[i]["out"], dtype=np.float32),
                SHARD_ROWS * i, axis=1) if i
        else np.asarray(res.results[i]["out"], dtype=np.float32)
        for i in range(N_CORES)
    ]
    out = np.concatenate(parts, axis=0)
    return out.reshape(B, S, D_OUT)
